# revision 1
# baseline (speedup 1.0000x reference)
"""Trainium2 Bass kernel for nn_BeliefModuleOld (segment_reduce).

Reference semantics per batch element b and treat type tt:
  valid[t] = (vision[b,t] != 0) and (max(visible_treats[b,t,tt,0:5]) > 0.5)
  out[b,tt,:] = visible_treats[b, last valid t, tt, :]  (or [0,0,0,0,0,1] if none)

Strategy: pure data-parallel over batch, 8 cores. Layout A: batch elements
live on SBUF partitions (P=125 used) and along the free dim (F per
partition). Per tile:
  - DMA x [P, F,5,2,6] f32 and v [P, F,5] i32 (contiguous per partition)
  - hm[t,tt] = max over d<5 of x (tensor_max tree on DVE)
  - valid = (hm > 0.5) * vision  (scalar_tensor_tensor)
  - out initialized to the default vector, then for t=0..4 ascending
    copy_predicated(out, valid[t] broadcast over d, x[t]) -- last valid wins
  - DMA out [P, F,2,6]
"""

import numpy as np

import concourse.bass as bass
import concourse.bacc as bacc
import concourse.tile as tile
from concourse import mybir
from concourse.alu_op_type import AluOpType
from concourse.bass_utils import run_bass_kernel_spmd

B, T, NT, D = 1_000_000, 5, 2, 6
NCORES = 8
BC = B // NCORES  # 125,000 per core
P = 125           # SBUF partitions used
F = 125           # batch elements per partition per tile
NTILES = BC // (P * F)  # 8 tiles, exact


def _copy_predicated(eng, out, mask, data):
    # Same as BassVectorEngine.copy_predicated but with opt=False lowering so
    # the three operand APs keep identical [p, f, nt, d] structure (the
    # broadcast mask AP cannot merge dims; unoptimized APs keep the sim's
    # np.where shapes aligned and the HW element streams in lockstep).
    return eng.add_instruction(
        mybir.InstCopyPredicated(
            name=f"I-{eng.bass.next_id()}",
            ins=[eng.lower_ap(mask, opt=False), eng.lower_ap(data, opt=False)],
            outs=[eng.lower_ap(out, opt=False)],
        )
    )


def build_nc(bc=BC, p=P, f=F, reps=1, mode="full", ring="coarsea"):
    ntiles = bc // (p * f)
    assert p * f * ntiles == bc, (bc, p, f)
    nc = bacc.Bacc("TRN2", target_bir_lowering=False)

    x = nc.dram_tensor("x", [bc, T, NT, D], mybir.dt.float32, kind="ExternalInput")
    v = nc.dram_tensor("v", [bc, T], mybir.dt.int32, kind="ExternalInput")
    o = nc.dram_tensor("o", [bc, NT, D], mybir.dt.float32, kind="ExternalOutput")

    # [ntiles, p, per-partition-contiguous block]
    xr = x[:].rearrange("(n p f) t nt d -> n p (f t nt d)", p=p, f=f)
    vr = v[:].rearrange("(n p f) t -> n p (f t)", p=p, f=f)
    orr = o[:].rearrange("(n p f) nt d -> n p (f nt d)", p=p, f=f)

    fdt = mybir.dt.float32

    if ring.startswith("coarse"):
        # One x-load feeds two compute subtiles: 4x 7.3MB loads instead of
        # 8x 3.66MB. vact ring split; in-place max tree to fit SBUF.
        # ring="coarseg": x-loads via the SWDGE (gpsimd) path instead of HWDGE.
        # ring="coarsea": coarseg + output init via ACT copy from a const tile
        # (keeps GPSIMD off the per-subtile critical path).
        xld = nc.sync if ring == "coarse" else nc.gpsimd
        f2 = 2 * f
        nload = bc // (p * f2)
        assert p * f2 * nload == bc
        xr2 = x[:].rearrange("(n p f) t nt d -> n p f t nt d", p=p, f=f2)
        vr2 = v[:].rearrange("(n p f) t -> n p (f t)", p=p, f=f2)
        or2 = o[:].rearrange("(n p f) nt d -> n p (f nt d)", p=p, f=f2)
        with tile.TileContext(nc) as tc:
            with (
                tc.tile_pool(name="xs", bufs=2) as xpool,
                tc.tile_pool(name="vs", bufs=3) as vpool,
                tc.tile_pool(name="os", bufs=3) as opool,
                tc.tile_pool(name="wk", bufs=2) as wpool,
            ):
                dflt = None
                if ring == "coarsea":
                    dflt = wpool.tile([p, NT, D], fdt, tag="dflt", bufs=1)
                    nc.gpsimd.memset(dflt[:, :, 0:5], 0.0)
                    nc.gpsimd.memset(dflt[:, :, 5:6], 1.0)
                for it in range(reps * nload * 2):
                    k, j = (it // 2) % nload, it % 2
                    if j == 0:
                        xb = xpool.tile([p, f2, T, NT, D], fdt, tag="x")
                        xbf = xb[:].rearrange("p f t nt d -> p (f t nt d)")
                        xsrc = xr2[k].rearrange("p f t nt d -> p (f t nt d)")
                        if ring == "coarsedual":
                            # halve across the two independent DGE feeders:
                            # HWDGE (sync RTL) and SWDGE (gpsimd Q7)
                            h = p // 2
                            nc.sync.dma_start(out=xbf[0:h, :], in_=xsrc[0:h, :])
                            nc.gpsimd.dma_start(out=xbf[h:p, :], in_=xsrc[h:p, :])
                        else:
                            xld.dma_start(out=xbf, in_=xsrc)
                    if mode == "dma":
                        # bench: same loads + a same-sized contiguous store
                        vt = vpool.tile([p, f, T], mybir.dt.int32, tag="v")
                        nc.scalar.dma_start(
                            out=vt[:].rearrange("p f t -> p (f t)"),
                            in_=vr2[k][:, j * f * T : (j + 1) * f * T],
                        )
                        nc.scalar.dma_start(
                            out=or2[k][:, j * f * NT * D : (j + 1) * f * NT * D],
                            in_=xb[:].rearrange("p f t nt d -> p (f t nt d)")[
                                :, 0 : f * NT * D
                            ],
                        )
                        continue
                    xt = xb[:, j * f : (j + 1) * f]
                    vt = vpool.tile([p, f, T], mybir.dt.int32, tag="v")
                    ot = opool.tile([p, f, NT, D], fdt, tag="o")
                    nc.scalar.dma_start(
                        out=vt[:].rearrange("p f t -> p (f t)"),
                        in_=vr2[k][:, j * f * T : (j + 1) * f * T],
                    )
                    if ring == "coarsea":
                        nc.scalar.copy(
                            ot[:],
                            dflt[:].unsqueeze(1).broadcast_to((p, f, NT, D)),
                        )
                    else:
                        nc.gpsimd.memset(ot[:, :, :, 0:5], 0.0)
                        nc.gpsimd.memset(ot[:, :, :, 5:6], 1.0)
                    a = wpool.tile([p, f, T, NT], fdt, tag="a")
                    bt = wpool.tile([p, f, T, NT], fdt, tag="b")
                    valid = wpool.tile([p, f, T, NT], mybir.dt.uint8, tag="valid")
                    nc.vector.tensor_max(a[:], xt[:, :, :, :, 0], xt[:, :, :, :, 1])
                    nc.vector.tensor_max(bt[:], xt[:, :, :, :, 2], xt[:, :, :, :, 3])
                    nc.vector.tensor_max(a[:], a[:], bt[:])
                    nc.vector.tensor_max(a[:], a[:], xt[:, :, :, :, 4])
                    vb = vt[:].unsqueeze(3).broadcast_to((p, f, T, NT))
                    nc.vector.scalar_tensor_tensor(
                        out=valid[:], in0=a[:], scalar=0.5, in1=vb,
                        op0=AluOpType.is_gt, op1=AluOpType.mult,
                    )
                    for t in range(T):
                        mask = (
                            valid[:, :, t, :]
                            .unsqueeze(3)
                            .broadcast_to((p, f, NT, D))
                        )
                        _copy_predicated(nc.vector, ot[:], mask, xt[:, :, t, :, :])
                    nc.scalar.dma_start(
                        out=or2[k][:, j * f * NT * D : (j + 1) * f * NT * D],
                        in_=ot[:].rearrange("p f nt d -> p (f nt d)"),
                    )
        nc.compile()
        return nc

    if mode.startswith("load128"):
        # pure-load microbench: x as [128, chunk] tiles from the flat region
        q = 244 if mode == "load128big" else 122
        n128 = (bc // (128 * q))
        xl = x[0 : n128 * 128 * q].rearrange(
            "(n p q) t nt d -> n p (q t nt d)", p=128, q=q
        )
        with tile.TileContext(nc) as tc:
            with tc.tile_pool(name="xs", bufs=2) as xpool:
                for it in range(reps * n128):
                    i = it % n128
                    xt = xpool.tile([128, q * T * NT * D], fdt, tag="x")
                    nc.sync.dma_start(out=xt[:], in_=xl[i])
        nc.compile()
        return nc

    with tile.TileContext(nc) as tc:
        with (
            tc.tile_pool(name="xs", bufs=3) as xpool,
            tc.tile_pool(name="vs", bufs=3) as vpool,
            tc.tile_pool(name="os", bufs=3) as opool,
            tc.tile_pool(name="wk", bufs=2) as wpool,
        ):
            vbig = None
            if ring == "vpre":
                # preload ALL vision data once (2.5 MB) so the steady state
                # has only the big x-loads and the output stores
                vbig = vpool.tile([p, ntiles, f, T], mybir.dt.int32, tag="vbig")
                nc.scalar.dma_start(
                    out=vbig[:].rearrange("p n f t -> p n (f t)"),
                    in_=v[:].rearrange("(n p f) t -> p n (f t)", p=p, f=f),
                )
            if mode == "compute":
                # bench mode: load one tile, run the compute chain reps*ntiles
                # times on resident tiles, store once.
                xt = xpool.tile([p, f, T, NT, D], fdt, tag="x")
                vt = vpool.tile([p, f, T], mybir.dt.int32, tag="v")
                ot = opool.tile([p, f, NT, D], fdt, tag="o")
                nc.sync.dma_start(
                    out=xt[:].rearrange("p f t nt d -> p (f t nt d)"), in_=xr[0]
                )
                nc.sync.dma_start(out=vt[:].rearrange("p f t -> p (f t)"), in_=vr[0])

            for it, i in enumerate(
                [i for _ in range(reps) for i in range(ntiles)]
            ):
                if mode != "compute":
                    xt = xpool.tile([p, f, T, NT, D], fdt, tag="x")
                    ot = opool.tile([p, f, NT, D], fdt, tag="o")
                    # ring="fixed": loads on SP ring, stores on ACT ring.
                    # ring="alt": alternate per tile. ring="split": halve the
                    # x-load across both rings. ring="vact": v-load on ACT.
                    # ring="vpre": vision preloaded once before the loop.
                    xtf = xt[:].rearrange("p f t nt d -> p (f t nt d)")
                    if ring == "vpre":
                        vt = None
                    else:
                        vt = vpool.tile([p, f, T], mybir.dt.int32, tag="v")
                        vtf = vt[:].rearrange("p f t -> p (f t)")
                    if ring == "split":
                        h = p // 2
                        nc.sync.dma_start(out=xtf[0:h, :], in_=xr[i][0:h, :])
                        nc.scalar.dma_start(out=xtf[h:p, :], in_=xr[i][h:p, :])
                        nc.sync.dma_start(out=vtf, in_=vr[i])
                    else:
                        ldeng = (
                            nc.sync if (ring != "alt" or it % 2 == 0) else nc.scalar
                        )
                        ldeng.dma_start(out=xtf, in_=xr[i])
                        if vt is not None:
                            veng = nc.scalar if ring == "vact" else nc.sync
                            veng.dma_start(out=vtf, in_=vr[i])
                if mode == "dma":
                    # bench mode: DMA traffic only; store a contiguous chunk
                    # of the x tile with the same shape as the real output
                    xflat = xt[:].rearrange("p f t nt d -> p (f t nt d)")
                    nc.scalar.dma_start(out=orr[i], in_=xflat[:, 0 : f * NT * D])
                    continue
                if mode == "load":
                    continue  # bench mode: loads only

                # out = default = [0,0,0,0,0,1]
                nc.gpsimd.memset(ot[:, :, :, 0:5], 0.0)
                nc.gpsimd.memset(ot[:, :, :, 5:6], 1.0)

                a = wpool.tile([p, f, T, NT], fdt, tag="a")
                bt = wpool.tile([p, f, T, NT], fdt, tag="b")
                c = wpool.tile([p, f, T, NT], fdt, tag="c")
                hm = wpool.tile([p, f, T, NT], fdt, tag="hm")
                # uint8: walrus requires an integer mask dtype for CopyPredicated
                valid = wpool.tile([p, f, T, NT], mybir.dt.uint8, tag="valid")

                nc.vector.tensor_max(a[:], xt[:, :, :, :, 0], xt[:, :, :, :, 1])
                nc.vector.tensor_max(bt[:], xt[:, :, :, :, 2], xt[:, :, :, :, 3])
                nc.vector.tensor_max(c[:], a[:], bt[:])
                nc.vector.tensor_max(hm[:], c[:], xt[:, :, :, :, 4])

                # valid = (hm > 0.5) * vision, vision broadcast over tt
                # (DVE: walrus rejects TensorScalarPtr/TensorTensor on Pool)
                if ring == "vpre" and mode != "compute":
                    vb = vbig[:, i, :, :].unsqueeze(3).broadcast_to((p, f, T, NT))
                else:
                    vb = vt[:].unsqueeze(3).broadcast_to((p, f, T, NT))
                nc.vector.scalar_tensor_tensor(
                    out=valid[:],
                    in0=hm[:],
                    scalar=0.5,
                    in1=vb,
                    op0=AluOpType.is_gt,
                    op1=AluOpType.mult,
                )

                # ascending t: last valid timestep wins
                for t in range(T):
                    mask = (
                        valid[:, :, t, :].unsqueeze(3).broadcast_to((p, f, NT, D))
                    )
                    _copy_predicated(nc.vector, ot[:], mask, xt[:, :, t, :, :])

                if mode != "compute":
                    # store on the opposite ring from this tile's x-load so it
                    # doesn't queue behind the next tile's big load
                    steng = nc.scalar if (ring != "alt" or it % 2 == 0) else nc.sync
                    steng.dma_start(
                        out=orr[i], in_=ot[:].rearrange("p f nt d -> p (f nt d)")
                    )

            if mode == "compute":
                nc.sync.dma_start(
                    out=orr[0], in_=ot[:].rearrange("p f nt d -> p (f nt d)")
                )

    nc.compile()
    return nc


_NC = None


def run_spmd(visible_treats: np.ndarray, vision: np.ndarray, **kwargs):
    global _NC
    if _NC is None:
        _NC = build_nc()
    if not kwargs.get("trace"):
        # NTFF profiling needs antenv.axon_hooks, absent in this container; a
        # stray BASS_TRACE env var would otherwise crash the run.
        import os

        os.environ.setdefault("BASS_NEVER_TRACE", "1")
    vt = np.ascontiguousarray(visible_treats, dtype=np.float32)
    vi = np.ascontiguousarray(vision, dtype=np.int32)
    in_maps = [
        {
            "x": vt[c * BC : (c + 1) * BC],
            "v": vi[c * BC : (c + 1) * BC],
        }
        for c in range(NCORES)
    ]
    return run_bass_kernel_spmd(_NC, in_maps, core_ids=list(range(NCORES)), **kwargs)


def kernel(visible_treats: np.ndarray, vision: np.ndarray) -> np.ndarray:
    res = run_spmd(visible_treats, vision)
    return np.concatenate([r["o"] for r in res.results], axis=0)



# revision 2
# speedup vs baseline: 1.8791x; 1.8791x over previous
"""Trainium2 Bass kernel for nn_BeliefModuleOld (segment_reduce).

Reference semantics per batch element b and treat type tt:
  valid[t] = (vision[b,t] != 0) and (max(visible_treats[b,t,tt,0:5]) > 0.5)
  out[b,tt,:] = visible_treats[b, last valid t, tt, :]  (or [0,0,0,0,0,1] if none)

Strategy: pure data-parallel over batch, 8 cores. Layout A: batch elements
live on SBUF partitions (P=125 used) and along the free dim (F per
partition). Per tile:
  - DMA x [P, F,5,2,6] f32 and v [P, F,5] i32 (contiguous per partition)
  - hm[t,tt] = max over d<5 of x (tensor_max tree on DVE)
  - valid = (hm > 0.5) * vision  (scalar_tensor_tensor)
  - out initialized to the default vector, then for t=0..4 ascending
    copy_predicated(out, valid[t] broadcast over d, x[t]) -- last valid wins
  - DMA out [P, F,2,6]
"""

import numpy as np

import concourse.bass as bass
import concourse.bacc as bacc
import concourse.tile as tile
from concourse import mybir
from concourse.alu_op_type import AluOpType
from concourse.bass_utils import run_bass_kernel_spmd

B, T, NT, D = 1_000_000, 5, 2, 6
NCORES = 8
BC = B // NCORES  # 125,000 per core
P = 125           # SBUF partitions used
F = 125           # batch elements per partition per tile
NTILES = BC // (P * F)  # 8 tiles, exact


def _copy_predicated(eng, out, mask, data):
    # Same as BassVectorEngine.copy_predicated but with opt=False lowering so
    # the three operand APs keep identical [p, f, nt, d] structure (the
    # broadcast mask AP cannot merge dims; unoptimized APs keep the sim's
    # np.where shapes aligned and the HW element streams in lockstep).
    return eng.add_instruction(
        mybir.InstCopyPredicated(
            name=f"I-{eng.bass.next_id()}",
            ins=[eng.lower_ap(mask, opt=False), eng.lower_ap(data, opt=False)],
            outs=[eng.lower_ap(out, opt=False)],
        )
    )


def build_nc(bc=BC, p=P, f=F, reps=1, mode="full", ring="coarsea"):
    ntiles = bc // (p * f)
    assert p * f * ntiles == bc, (bc, p, f)
    nc = bacc.Bacc("TRN2", target_bir_lowering=False)

    x = nc.dram_tensor("x", [bc, T, NT, D], mybir.dt.float32, kind="ExternalInput")
    v = nc.dram_tensor("v", [bc, T], mybir.dt.int32, kind="ExternalInput")
    o = nc.dram_tensor("o", [bc, NT, D], mybir.dt.float32, kind="ExternalOutput")

    # [ntiles, p, per-partition-contiguous block]
    xr = x[:].rearrange("(n p f) t nt d -> n p (f t nt d)", p=p, f=f)
    vr = v[:].rearrange("(n p f) t -> n p (f t)", p=p, f=f)
    orr = o[:].rearrange("(n p f) nt d -> n p (f nt d)", p=p, f=f)

    fdt = mybir.dt.float32

    if ring.startswith("rr"):
        # Round-robin WHOLE x-tile loads across nq independent DMA queues
        # (each queue sustains only ~110-130 GB/s; splitting one load's
        # partition range across queues — coarsedual — is 2x WORSE, but
        # giving different queues different complete tiles scales BW).
        # v-loads and o-stores stay on the scalar(ACT) queue.
        nq = int(ring[2:])
        engs = [nc.gpsimd, nc.sync, nc.tensor, nc.vector][:nq]
        fl = 250 if nq <= 2 else 125
        nload = bc // (p * fl)
        assert p * fl * nload == bc
        xr2 = x[:].rearrange("(n p f) t nt d -> n p (f t nt d)", p=p, f=fl)
        vr2 = v[:].rearrange("(n p f) t -> n p (f t)", p=p, f=fl)
        or2 = o[:].rearrange("(n p f) nt d -> n p (f nt d)", p=p, f=fl)
        xbufs = 2 if nq <= 2 else nq
        with tile.TileContext(nc) as tc:
            with (
                tc.tile_pool(name="xs", bufs=xbufs) as xpool,
                tc.tile_pool(name="vs", bufs=2) as vpool,
                tc.tile_pool(name="os", bufs=2) as opool,
                tc.tile_pool(name="wk", bufs=2) as wpool,
            ):
                dflt = wpool.tile([p, NT, D], fdt, tag="dflt", bufs=1)
                nc.gpsimd.memset(dflt[:, :, 0:5], 0.0)
                nc.gpsimd.memset(dflt[:, :, 5:6], 1.0)
                for it in range(reps * nload):
                    k = it % nload
                    xt = xpool.tile([p, fl, T, NT, D], fdt, tag="x")
                    engs[it % nq].dma_start(
                        out=xt[:].rearrange("p f t nt d -> p (f t nt d)"),
                        in_=xr2[k],
                    )
                    if mode == "dma":
                        vt = vpool.tile([p, fl, T], mybir.dt.int32, tag="v")
                        nc.scalar.dma_start(
                            out=vt[:].rearrange("p f t -> p (f t)"), in_=vr2[k]
                        )
                        nc.scalar.dma_start(
                            out=or2[k],
                            in_=xt[:].rearrange("p f t nt d -> p (f t nt d)")[
                                :, 0 : fl * NT * D
                            ],
                        )
                        continue
                    vt = vpool.tile([p, fl, T], mybir.dt.int32, tag="v")
                    ot = opool.tile([p, fl, NT, D], fdt, tag="o")
                    nc.scalar.dma_start(
                        out=vt[:].rearrange("p f t -> p (f t)"), in_=vr2[k]
                    )
                    nc.scalar.copy(
                        ot[:], dflt[:].unsqueeze(1).broadcast_to((p, fl, NT, D))
                    )
                    a = wpool.tile([p, fl, T, NT], fdt, tag="a")
                    bt = wpool.tile([p, fl, T, NT], fdt, tag="b")
                    valid = wpool.tile([p, fl, T, NT], mybir.dt.uint8, tag="valid")
                    nc.vector.tensor_max(a[:], xt[:, :, :, :, 0], xt[:, :, :, :, 1])
                    nc.vector.tensor_max(bt[:], xt[:, :, :, :, 2], xt[:, :, :, :, 3])
                    nc.vector.tensor_max(a[:], a[:], bt[:])
                    nc.vector.tensor_max(a[:], a[:], xt[:, :, :, :, 4])
                    vb = vt[:].unsqueeze(3).broadcast_to((p, fl, T, NT))
                    nc.vector.scalar_tensor_tensor(
                        out=valid[:], in0=a[:], scalar=0.5, in1=vb,
                        op0=AluOpType.is_gt, op1=AluOpType.mult,
                    )
                    for t in range(T):
                        mask = (
                            valid[:, :, t, :]
                            .unsqueeze(3)
                            .broadcast_to((p, fl, NT, D))
                        )
                        _copy_predicated(nc.vector, ot[:], mask, xt[:, :, t, :, :])
                    nc.scalar.dma_start(
                        out=or2[k], in_=ot[:].rearrange("p f nt d -> p (f nt d)")
                    )
        nc.compile()
        return nc

    if ring.startswith("coarse"):
        # One x-load feeds two compute subtiles: 4x 7.3MB loads instead of
        # 8x 3.66MB. vact ring split; in-place max tree to fit SBUF.
        # ring="coarseg": x-loads via the SWDGE (gpsimd) path instead of HWDGE.
        # ring="coarsea": coarseg + output init via ACT copy from a const tile
        # (keeps GPSIMD off the per-subtile critical path).
        xld = nc.sync if ring == "coarse" else nc.gpsimd
        f2 = 2 * f
        nload = bc // (p * f2)
        assert p * f2 * nload == bc
        xr2 = x[:].rearrange("(n p f) t nt d -> n p f t nt d", p=p, f=f2)
        vr2 = v[:].rearrange("(n p f) t -> n p (f t)", p=p, f=f2)
        or2 = o[:].rearrange("(n p f) nt d -> n p (f nt d)", p=p, f=f2)
        with tile.TileContext(nc) as tc:
            with (
                tc.tile_pool(name="xs", bufs=2) as xpool,
                tc.tile_pool(name="vs", bufs=3) as vpool,
                tc.tile_pool(name="os", bufs=3) as opool,
                tc.tile_pool(name="wk", bufs=2) as wpool,
            ):
                dflt = None
                if ring == "coarsea":
                    dflt = wpool.tile([p, NT, D], fdt, tag="dflt", bufs=1)
                    nc.gpsimd.memset(dflt[:, :, 0:5], 0.0)
                    nc.gpsimd.memset(dflt[:, :, 5:6], 1.0)
                for it in range(reps * nload * 2):
                    k, j = (it // 2) % nload, it % 2
                    if j == 0:
                        xb = xpool.tile([p, f2, T, NT, D], fdt, tag="x")
                        xbf = xb[:].rearrange("p f t nt d -> p (f t nt d)")
                        xsrc = xr2[k].rearrange("p f t nt d -> p (f t nt d)")
                        if ring == "coarsedual":
                            # halve across the two independent DGE feeders:
                            # HWDGE (sync RTL) and SWDGE (gpsimd Q7)
                            h = p // 2
                            nc.sync.dma_start(out=xbf[0:h, :], in_=xsrc[0:h, :])
                            nc.gpsimd.dma_start(out=xbf[h:p, :], in_=xsrc[h:p, :])
                        else:
                            xld.dma_start(out=xbf, in_=xsrc)
                    if mode == "dma":
                        # bench: same loads + a same-sized contiguous store
                        vt = vpool.tile([p, f, T], mybir.dt.int32, tag="v")
                        nc.scalar.dma_start(
                            out=vt[:].rearrange("p f t -> p (f t)"),
                            in_=vr2[k][:, j * f * T : (j + 1) * f * T],
                        )
                        nc.scalar.dma_start(
                            out=or2[k][:, j * f * NT * D : (j + 1) * f * NT * D],
                            in_=xb[:].rearrange("p f t nt d -> p (f t nt d)")[
                                :, 0 : f * NT * D
                            ],
                        )
                        continue
                    xt = xb[:, j * f : (j + 1) * f]
                    vt = vpool.tile([p, f, T], mybir.dt.int32, tag="v")
                    ot = opool.tile([p, f, NT, D], fdt, tag="o")
                    nc.scalar.dma_start(
                        out=vt[:].rearrange("p f t -> p (f t)"),
                        in_=vr2[k][:, j * f * T : (j + 1) * f * T],
                    )
                    if ring == "coarsea":
                        nc.scalar.copy(
                            ot[:],
                            dflt[:].unsqueeze(1).broadcast_to((p, f, NT, D)),
                        )
                    else:
                        nc.gpsimd.memset(ot[:, :, :, 0:5], 0.0)
                        nc.gpsimd.memset(ot[:, :, :, 5:6], 1.0)
                    a = wpool.tile([p, f, T, NT], fdt, tag="a")
                    bt = wpool.tile([p, f, T, NT], fdt, tag="b")
                    valid = wpool.tile([p, f, T, NT], mybir.dt.uint8, tag="valid")
                    nc.vector.tensor_max(a[:], xt[:, :, :, :, 0], xt[:, :, :, :, 1])
                    nc.vector.tensor_max(bt[:], xt[:, :, :, :, 2], xt[:, :, :, :, 3])
                    nc.vector.tensor_max(a[:], a[:], bt[:])
                    nc.vector.tensor_max(a[:], a[:], xt[:, :, :, :, 4])
                    vb = vt[:].unsqueeze(3).broadcast_to((p, f, T, NT))
                    nc.vector.scalar_tensor_tensor(
                        out=valid[:], in0=a[:], scalar=0.5, in1=vb,
                        op0=AluOpType.is_gt, op1=AluOpType.mult,
                    )
                    for t in range(T):
                        mask = (
                            valid[:, :, t, :]
                            .unsqueeze(3)
                            .broadcast_to((p, f, NT, D))
                        )
                        _copy_predicated(nc.vector, ot[:], mask, xt[:, :, t, :, :])
                    nc.scalar.dma_start(
                        out=or2[k][:, j * f * NT * D : (j + 1) * f * NT * D],
                        in_=ot[:].rearrange("p f nt d -> p (f nt d)"),
                    )
        nc.compile()
        return nc

    if mode.startswith("load128"):
        # pure-load microbench: x as [128, chunk] tiles from the flat region
        q = 244 if mode == "load128big" else 122
        n128 = (bc // (128 * q))
        xl = x[0 : n128 * 128 * q].rearrange(
            "(n p q) t nt d -> n p (q t nt d)", p=128, q=q
        )
        with tile.TileContext(nc) as tc:
            with tc.tile_pool(name="xs", bufs=2) as xpool:
                for it in range(reps * n128):
                    i = it % n128
                    xt = xpool.tile([128, q * T * NT * D], fdt, tag="x")
                    nc.sync.dma_start(out=xt[:], in_=xl[i])
        nc.compile()
        return nc

    with tile.TileContext(nc) as tc:
        with (
            tc.tile_pool(name="xs", bufs=3) as xpool,
            tc.tile_pool(name="vs", bufs=3) as vpool,
            tc.tile_pool(name="os", bufs=3) as opool,
            tc.tile_pool(name="wk", bufs=2) as wpool,
        ):
            vbig = None
            if ring == "vpre":
                # preload ALL vision data once (2.5 MB) so the steady state
                # has only the big x-loads and the output stores
                vbig = vpool.tile([p, ntiles, f, T], mybir.dt.int32, tag="vbig")
                nc.scalar.dma_start(
                    out=vbig[:].rearrange("p n f t -> p n (f t)"),
                    in_=v[:].rearrange("(n p f) t -> p n (f t)", p=p, f=f),
                )
            if mode == "compute":
                # bench mode: load one tile, run the compute chain reps*ntiles
                # times on resident tiles, store once.
                xt = xpool.tile([p, f, T, NT, D], fdt, tag="x")
                vt = vpool.tile([p, f, T], mybir.dt.int32, tag="v")
                ot = opool.tile([p, f, NT, D], fdt, tag="o")
                nc.sync.dma_start(
                    out=xt[:].rearrange("p f t nt d -> p (f t nt d)"), in_=xr[0]
                )
                nc.sync.dma_start(out=vt[:].rearrange("p f t -> p (f t)"), in_=vr[0])

            for it, i in enumerate(
                [i for _ in range(reps) for i in range(ntiles)]
            ):
                if mode != "compute":
                    xt = xpool.tile([p, f, T, NT, D], fdt, tag="x")
                    ot = opool.tile([p, f, NT, D], fdt, tag="o")
                    # ring="fixed": loads on SP ring, stores on ACT ring.
                    # ring="alt": alternate per tile. ring="split": halve the
                    # x-load across both rings. ring="vact": v-load on ACT.
                    # ring="vpre": vision preloaded once before the loop.
                    xtf = xt[:].rearrange("p f t nt d -> p (f t nt d)")
                    if ring == "vpre":
                        vt = None
                    else:
                        vt = vpool.tile([p, f, T], mybir.dt.int32, tag="v")
                        vtf = vt[:].rearrange("p f t -> p (f t)")
                    if ring == "split":
                        h = p // 2
                        nc.sync.dma_start(out=xtf[0:h, :], in_=xr[i][0:h, :])
                        nc.scalar.dma_start(out=xtf[h:p, :], in_=xr[i][h:p, :])
                        nc.sync.dma_start(out=vtf, in_=vr[i])
                    else:
                        ldeng = (
                            nc.sync if (ring != "alt" or it % 2 == 0) else nc.scalar
                        )
                        ldeng.dma_start(out=xtf, in_=xr[i])
                        if vt is not None:
                            veng = nc.scalar if ring == "vact" else nc.sync
                            veng.dma_start(out=vtf, in_=vr[i])
                if mode == "dma":
                    # bench mode: DMA traffic only; store a contiguous chunk
                    # of the x tile with the same shape as the real output
                    xflat = xt[:].rearrange("p f t nt d -> p (f t nt d)")
                    nc.scalar.dma_start(out=orr[i], in_=xflat[:, 0 : f * NT * D])
                    continue
                if mode == "load":
                    continue  # bench mode: loads only

                # out = default = [0,0,0,0,0,1]
                nc.gpsimd.memset(ot[:, :, :, 0:5], 0.0)
                nc.gpsimd.memset(ot[:, :, :, 5:6], 1.0)

                a = wpool.tile([p, f, T, NT], fdt, tag="a")
                bt = wpool.tile([p, f, T, NT], fdt, tag="b")
                c = wpool.tile([p, f, T, NT], fdt, tag="c")
                hm = wpool.tile([p, f, T, NT], fdt, tag="hm")
                # uint8: walrus requires an integer mask dtype for CopyPredicated
                valid = wpool.tile([p, f, T, NT], mybir.dt.uint8, tag="valid")

                nc.vector.tensor_max(a[:], xt[:, :, :, :, 0], xt[:, :, :, :, 1])
                nc.vector.tensor_max(bt[:], xt[:, :, :, :, 2], xt[:, :, :, :, 3])
                nc.vector.tensor_max(c[:], a[:], bt[:])
                nc.vector.tensor_max(hm[:], c[:], xt[:, :, :, :, 4])

                # valid = (hm > 0.5) * vision, vision broadcast over tt
                # (DVE: walrus rejects TensorScalarPtr/TensorTensor on Pool)
                if ring == "vpre" and mode != "compute":
                    vb = vbig[:, i, :, :].unsqueeze(3).broadcast_to((p, f, T, NT))
                else:
                    vb = vt[:].unsqueeze(3).broadcast_to((p, f, T, NT))
                nc.vector.scalar_tensor_tensor(
                    out=valid[:],
                    in0=hm[:],
                    scalar=0.5,
                    in1=vb,
                    op0=AluOpType.is_gt,
                    op1=AluOpType.mult,
                )

                # ascending t: last valid timestep wins
                for t in range(T):
                    mask = (
                        valid[:, :, t, :].unsqueeze(3).broadcast_to((p, f, NT, D))
                    )
                    _copy_predicated(nc.vector, ot[:], mask, xt[:, :, t, :, :])

                if mode != "compute":
                    # store on the opposite ring from this tile's x-load so it
                    # doesn't queue behind the next tile's big load
                    steng = nc.scalar if (ring != "alt" or it % 2 == 0) else nc.sync
                    steng.dma_start(
                        out=orr[i], in_=ot[:].rearrange("p f nt d -> p (f nt d)")
                    )

            if mode == "compute":
                nc.sync.dma_start(
                    out=orr[0], in_=ot[:].rearrange("p f nt d -> p (f nt d)")
                )

    nc.compile()
    return nc


_NC = None


def run_spmd(visible_treats: np.ndarray, vision: np.ndarray, **kwargs):
    global _NC
    if _NC is None:
        _NC = build_nc()
    if not kwargs.get("trace"):
        # NTFF profiling needs antenv.axon_hooks, absent in this container; a
        # stray BASS_TRACE env var would otherwise crash the run.
        import os

        os.environ.setdefault("BASS_NEVER_TRACE", "1")
    vt = np.ascontiguousarray(visible_treats, dtype=np.float32)
    vi = np.ascontiguousarray(vision, dtype=np.int32)
    in_maps = [
        {
            "x": vt[c * BC : (c + 1) * BC],
            "v": vi[c * BC : (c + 1) * BC],
        }
        for c in range(NCORES)
    ]
    return run_bass_kernel_spmd(_NC, in_maps, core_ids=list(range(NCORES)), **kwargs)


def kernel(visible_treats: np.ndarray, vision: np.ndarray) -> np.ndarray:
    res = run_spmd(visible_treats, vision)
    return np.concatenate([r["o"] for r in res.results], axis=0)



# revision 3
# speedup vs baseline: 2.0034x; 1.0661x over previous
"""Trainium2 Bass kernel for nn_BeliefModuleOld (segment_reduce).

Reference semantics per batch element b and treat type tt:
  valid[t] = (vision[b,t] != 0) and (max(visible_treats[b,t,tt,0:5]) > 0.5)
  out[b,tt,:] = visible_treats[b, last valid t, tt, :]  (or [0,0,0,0,0,1] if none)

Strategy: pure data-parallel over batch, 8 cores. Layout A: batch elements
live on SBUF partitions (P=125 used) and along the free dim (F per
partition). Per tile:
  - DMA x [P, F,5,2,6] f32 and v [P, F,5] i32 (contiguous per partition)
  - hm[t,tt] = max over d<5 of x (tensor_max tree on DVE)
  - valid = (hm > 0.5) * vision  (scalar_tensor_tensor)
  - out initialized to the default vector, then for t=0..4 ascending
    copy_predicated(out, valid[t] broadcast over d, x[t]) -- last valid wins
  - DMA out [P, F,2,6]
"""

import numpy as np

import concourse.bass as bass
import concourse.bacc as bacc
import concourse.tile as tile
from concourse import mybir
from concourse.alu_op_type import AluOpType
from concourse.bass_utils import run_bass_kernel_spmd

B, T, NT, D = 1_000_000, 5, 2, 6
NCORES = 8
BC = B // NCORES  # 125,000 per core
P = 125           # SBUF partitions used
F = 125           # batch elements per partition per tile
NTILES = BC // (P * F)  # 8 tiles, exact


def _copy_predicated(eng, out, mask, data):
    # Same as BassVectorEngine.copy_predicated but with opt=False lowering so
    # the three operand APs keep identical [p, f, nt, d] structure (the
    # broadcast mask AP cannot merge dims; unoptimized APs keep the sim's
    # np.where shapes aligned and the HW element streams in lockstep).
    return eng.add_instruction(
        mybir.InstCopyPredicated(
            name=f"I-{eng.bass.next_id()}",
            ins=[eng.lower_ap(mask, opt=False), eng.lower_ap(data, opt=False)],
            outs=[eng.lower_ap(out, opt=False)],
        )
    )


def build_nc(bc=BC, p=P, f=F, reps=1, mode="full", ring="coarsea"):
    ntiles = bc // (p * f)
    assert p * f * ntiles == bc, (bc, p, f)
    nc = bacc.Bacc("TRN2", target_bir_lowering=False)

    x = nc.dram_tensor("x", [bc, T, NT, D], mybir.dt.float32, kind="ExternalInput")
    v = nc.dram_tensor("v", [bc, T], mybir.dt.int32, kind="ExternalInput")
    o = nc.dram_tensor("o", [bc, NT, D], mybir.dt.float32, kind="ExternalOutput")

    # [ntiles, p, per-partition-contiguous block]
    xr = x[:].rearrange("(n p f) t nt d -> n p (f t nt d)", p=p, f=f)
    vr = v[:].rearrange("(n p f) t -> n p (f t)", p=p, f=f)
    orr = o[:].rearrange("(n p f) nt d -> n p (f nt d)", p=p, f=f)

    fdt = mybir.dt.float32

    if ring.startswith("rr"):
        # Round-robin WHOLE x-tile loads across nq independent DMA queues
        # (each queue sustains only ~110-130 GB/s; splitting one load's
        # partition range across queues — coarsedual — is 2x WORSE, but
        # giving different queues different complete tiles scales BW).
        # v-loads and o-stores stay on the scalar(ACT) queue.
        nq = int(ring[2:])
        engs = [nc.gpsimd, nc.sync, nc.tensor, nc.vector][:nq]
        fl = 250 if nq <= 2 else 125
        nload = bc // (p * fl)
        assert p * fl * nload == bc
        xr2 = x[:].rearrange("(n p f) t nt d -> n p (f t nt d)", p=p, f=fl)
        vr2 = v[:].rearrange("(n p f) t -> n p (f t)", p=p, f=fl)
        or2 = o[:].rearrange("(n p f) nt d -> n p (f nt d)", p=p, f=fl)
        xbufs = 2 if nq <= 2 else nq
        with tile.TileContext(nc) as tc:
            with (
                tc.tile_pool(name="xs", bufs=xbufs) as xpool,
                tc.tile_pool(name="vs", bufs=2) as vpool,
                tc.tile_pool(name="os", bufs=2) as opool,
                tc.tile_pool(name="wk", bufs=2) as wpool,
            ):
                dflt = wpool.tile([p, NT, D], fdt, tag="dflt", bufs=1)
                nc.gpsimd.memset(dflt[:, :, 0:5], 0.0)
                nc.gpsimd.memset(dflt[:, :, 5:6], 1.0)
                for it in range(reps * nload):
                    k = it % nload
                    xt = xpool.tile([p, fl, T, NT, D], fdt, tag="x")
                    engs[it % nq].dma_start(
                        out=xt[:].rearrange("p f t nt d -> p (f t nt d)"),
                        in_=xr2[k],
                    )
                    if mode == "loadrr":
                        continue
                    if mode == "dma":
                        vt = vpool.tile([p, fl, T], mybir.dt.int32, tag="v")
                        nc.scalar.dma_start(
                            out=vt[:].rearrange("p f t -> p (f t)"), in_=vr2[k]
                        )
                        nc.scalar.dma_start(
                            out=or2[k],
                            in_=xt[:].rearrange("p f t nt d -> p (f t nt d)")[
                                :, 0 : fl * NT * D
                            ],
                        )
                        continue
                    vt = vpool.tile([p, fl, T], mybir.dt.int32, tag="v")
                    ot = opool.tile([p, fl, NT, D], fdt, tag="o")
                    nc.scalar.dma_start(
                        out=vt[:].rearrange("p f t -> p (f t)"), in_=vr2[k]
                    )
                    nc.scalar.copy(
                        ot[:], dflt[:].unsqueeze(1).broadcast_to((p, fl, NT, D))
                    )
                    a = wpool.tile([p, fl, T, NT], fdt, tag="a")
                    bt = wpool.tile([p, fl, T, NT], fdt, tag="b")
                    valid = wpool.tile([p, fl, T, NT], mybir.dt.uint8, tag="valid")
                    nc.vector.tensor_max(a[:], xt[:, :, :, :, 0], xt[:, :, :, :, 1])
                    nc.vector.tensor_max(bt[:], xt[:, :, :, :, 2], xt[:, :, :, :, 3])
                    nc.vector.tensor_max(a[:], a[:], bt[:])
                    nc.vector.tensor_max(a[:], a[:], xt[:, :, :, :, 4])
                    vb = vt[:].unsqueeze(3).broadcast_to((p, fl, T, NT))
                    nc.vector.scalar_tensor_tensor(
                        out=valid[:], in0=a[:], scalar=0.5, in1=vb,
                        op0=AluOpType.is_gt, op1=AluOpType.mult,
                    )
                    for t in range(T):
                        mask = (
                            valid[:, :, t, :]
                            .unsqueeze(3)
                            .broadcast_to((p, fl, NT, D))
                        )
                        _copy_predicated(nc.vector, ot[:], mask, xt[:, :, t, :, :])
                    nc.scalar.dma_start(
                        out=or2[k], in_=ot[:].rearrange("p f nt d -> p (f nt d)")
                    )
        nc.compile()
        return nc

    if ring.startswith("coarse"):
        # One x-load feeds two compute subtiles: 4x 7.3MB loads instead of
        # 8x 3.66MB. vact ring split; in-place max tree to fit SBUF.
        # ring="coarseg": x-loads via the SWDGE (gpsimd) path instead of HWDGE.
        # ring="coarsea": coarseg + output init via ACT copy from a const tile
        # (keeps GPSIMD off the per-subtile critical path).
        xld = nc.sync if ring == "coarse" else nc.gpsimd
        f2 = 2 * f
        nload = bc // (p * f2)
        assert p * f2 * nload == bc
        xr2 = x[:].rearrange("(n p f) t nt d -> n p f t nt d", p=p, f=f2)
        vr2 = v[:].rearrange("(n p f) t -> n p (f t)", p=p, f=f2)
        or2 = o[:].rearrange("(n p f) nt d -> n p (f nt d)", p=p, f=f2)
        with tile.TileContext(nc) as tc:
            with (
                tc.tile_pool(name="xs", bufs=2) as xpool,
                tc.tile_pool(name="vs", bufs=3) as vpool,
                tc.tile_pool(name="os", bufs=3) as opool,
                tc.tile_pool(name="wk", bufs=2) as wpool,
            ):
                dflt = None
                if ring == "coarsea":
                    dflt = wpool.tile([p, NT, D], fdt, tag="dflt", bufs=1)
                    nc.gpsimd.memset(dflt[:, :, 0:5], 0.0)
                    nc.gpsimd.memset(dflt[:, :, 5:6], 1.0)
                for it in range(reps * nload * 2):
                    k, j = (it // 2) % nload, it % 2
                    if j == 0:
                        xb = xpool.tile([p, f2, T, NT, D], fdt, tag="x")
                        xbf = xb[:].rearrange("p f t nt d -> p (f t nt d)")
                        xsrc = xr2[k].rearrange("p f t nt d -> p (f t nt d)")
                        if ring == "coarsedual":
                            # halve across the two independent DGE feeders:
                            # HWDGE (sync RTL) and SWDGE (gpsimd Q7)
                            h = p // 2
                            nc.sync.dma_start(out=xbf[0:h, :], in_=xsrc[0:h, :])
                            nc.gpsimd.dma_start(out=xbf[h:p, :], in_=xsrc[h:p, :])
                        else:
                            xld.dma_start(out=xbf, in_=xsrc)
                    if mode == "dma":
                        # bench: same loads + a same-sized contiguous store
                        vt = vpool.tile([p, f, T], mybir.dt.int32, tag="v")
                        nc.scalar.dma_start(
                            out=vt[:].rearrange("p f t -> p (f t)"),
                            in_=vr2[k][:, j * f * T : (j + 1) * f * T],
                        )
                        nc.scalar.dma_start(
                            out=or2[k][:, j * f * NT * D : (j + 1) * f * NT * D],
                            in_=xb[:].rearrange("p f t nt d -> p (f t nt d)")[
                                :, 0 : f * NT * D
                            ],
                        )
                        continue
                    xt = xb[:, j * f : (j + 1) * f]
                    vt = vpool.tile([p, f, T], mybir.dt.int32, tag="v")
                    ot = opool.tile([p, f, NT, D], fdt, tag="o")
                    nc.scalar.dma_start(
                        out=vt[:].rearrange("p f t -> p (f t)"),
                        in_=vr2[k][:, j * f * T : (j + 1) * f * T],
                    )
                    if ring == "coarsea":
                        nc.scalar.copy(
                            ot[:],
                            dflt[:].unsqueeze(1).broadcast_to((p, f, NT, D)),
                        )
                    else:
                        nc.gpsimd.memset(ot[:, :, :, 0:5], 0.0)
                        nc.gpsimd.memset(ot[:, :, :, 5:6], 1.0)
                    a = wpool.tile([p, f, T, NT], fdt, tag="a")
                    bt = wpool.tile([p, f, T, NT], fdt, tag="b")
                    valid = wpool.tile([p, f, T, NT], mybir.dt.uint8, tag="valid")
                    nc.vector.tensor_max(a[:], xt[:, :, :, :, 0], xt[:, :, :, :, 1])
                    nc.vector.tensor_max(bt[:], xt[:, :, :, :, 2], xt[:, :, :, :, 3])
                    nc.vector.tensor_max(a[:], a[:], bt[:])
                    nc.vector.tensor_max(a[:], a[:], xt[:, :, :, :, 4])
                    vb = vt[:].unsqueeze(3).broadcast_to((p, f, T, NT))
                    nc.vector.scalar_tensor_tensor(
                        out=valid[:], in0=a[:], scalar=0.5, in1=vb,
                        op0=AluOpType.is_gt, op1=AluOpType.mult,
                    )
                    for t in range(T):
                        mask = (
                            valid[:, :, t, :]
                            .unsqueeze(3)
                            .broadcast_to((p, f, NT, D))
                        )
                        _copy_predicated(nc.vector, ot[:], mask, xt[:, :, t, :, :])
                    nc.scalar.dma_start(
                        out=or2[k][:, j * f * NT * D : (j + 1) * f * NT * D],
                        in_=ot[:].rearrange("p f nt d -> p (f nt d)"),
                    )
        nc.compile()
        return nc

    if mode.startswith("load128"):
        # pure-load microbench: x as [128, chunk] tiles from the flat region
        q = 244 if mode == "load128big" else 122
        n128 = (bc // (128 * q))
        xl = x[0 : n128 * 128 * q].rearrange(
            "(n p q) t nt d -> n p (q t nt d)", p=128, q=q
        )
        with tile.TileContext(nc) as tc:
            with tc.tile_pool(name="xs", bufs=2) as xpool:
                for it in range(reps * n128):
                    i = it % n128
                    xt = xpool.tile([128, q * T * NT * D], fdt, tag="x")
                    nc.sync.dma_start(out=xt[:], in_=xl[i])
        nc.compile()
        return nc

    with tile.TileContext(nc) as tc:
        with (
            tc.tile_pool(name="xs", bufs=3) as xpool,
            tc.tile_pool(name="vs", bufs=3) as vpool,
            tc.tile_pool(name="os", bufs=3) as opool,
            tc.tile_pool(name="wk", bufs=2) as wpool,
        ):
            vbig = None
            if ring == "vpre":
                # preload ALL vision data once (2.5 MB) so the steady state
                # has only the big x-loads and the output stores
                vbig = vpool.tile([p, ntiles, f, T], mybir.dt.int32, tag="vbig")
                nc.scalar.dma_start(
                    out=vbig[:].rearrange("p n f t -> p n (f t)"),
                    in_=v[:].rearrange("(n p f) t -> p n (f t)", p=p, f=f),
                )
            if mode == "compute":
                # bench mode: load one tile, run the compute chain reps*ntiles
                # times on resident tiles, store once.
                xt = xpool.tile([p, f, T, NT, D], fdt, tag="x")
                vt = vpool.tile([p, f, T], mybir.dt.int32, tag="v")
                ot = opool.tile([p, f, NT, D], fdt, tag="o")
                nc.sync.dma_start(
                    out=xt[:].rearrange("p f t nt d -> p (f t nt d)"), in_=xr[0]
                )
                nc.sync.dma_start(out=vt[:].rearrange("p f t -> p (f t)"), in_=vr[0])

            for it, i in enumerate(
                [i for _ in range(reps) for i in range(ntiles)]
            ):
                if mode != "compute":
                    xt = xpool.tile([p, f, T, NT, D], fdt, tag="x")
                    ot = opool.tile([p, f, NT, D], fdt, tag="o")
                    # ring="fixed": loads on SP ring, stores on ACT ring.
                    # ring="alt": alternate per tile. ring="split": halve the
                    # x-load across both rings. ring="vact": v-load on ACT.
                    # ring="vpre": vision preloaded once before the loop.
                    xtf = xt[:].rearrange("p f t nt d -> p (f t nt d)")
                    if ring == "vpre":
                        vt = None
                    else:
                        vt = vpool.tile([p, f, T], mybir.dt.int32, tag="v")
                        vtf = vt[:].rearrange("p f t -> p (f t)")
                    if ring == "split":
                        h = p // 2
                        nc.sync.dma_start(out=xtf[0:h, :], in_=xr[i][0:h, :])
                        nc.scalar.dma_start(out=xtf[h:p, :], in_=xr[i][h:p, :])
                        nc.sync.dma_start(out=vtf, in_=vr[i])
                    else:
                        ldeng = (
                            nc.sync if (ring != "alt" or it % 2 == 0) else nc.scalar
                        )
                        ldeng.dma_start(out=xtf, in_=xr[i])
                        if vt is not None:
                            veng = nc.scalar if ring == "vact" else nc.sync
                            veng.dma_start(out=vtf, in_=vr[i])
                if mode == "dma":
                    # bench mode: DMA traffic only; store a contiguous chunk
                    # of the x tile with the same shape as the real output
                    xflat = xt[:].rearrange("p f t nt d -> p (f t nt d)")
                    nc.scalar.dma_start(out=orr[i], in_=xflat[:, 0 : f * NT * D])
                    continue
                if mode == "load":
                    continue  # bench mode: loads only

                # out = default = [0,0,0,0,0,1]
                nc.gpsimd.memset(ot[:, :, :, 0:5], 0.0)
                nc.gpsimd.memset(ot[:, :, :, 5:6], 1.0)

                a = wpool.tile([p, f, T, NT], fdt, tag="a")
                bt = wpool.tile([p, f, T, NT], fdt, tag="b")
                c = wpool.tile([p, f, T, NT], fdt, tag="c")
                hm = wpool.tile([p, f, T, NT], fdt, tag="hm")
                # uint8: walrus requires an integer mask dtype for CopyPredicated
                valid = wpool.tile([p, f, T, NT], mybir.dt.uint8, tag="valid")

                nc.vector.tensor_max(a[:], xt[:, :, :, :, 0], xt[:, :, :, :, 1])
                nc.vector.tensor_max(bt[:], xt[:, :, :, :, 2], xt[:, :, :, :, 3])
                nc.vector.tensor_max(c[:], a[:], bt[:])
                nc.vector.tensor_max(hm[:], c[:], xt[:, :, :, :, 4])

                # valid = (hm > 0.5) * vision, vision broadcast over tt
                # (DVE: walrus rejects TensorScalarPtr/TensorTensor on Pool)
                if ring == "vpre" and mode != "compute":
                    vb = vbig[:, i, :, :].unsqueeze(3).broadcast_to((p, f, T, NT))
                else:
                    vb = vt[:].unsqueeze(3).broadcast_to((p, f, T, NT))
                nc.vector.scalar_tensor_tensor(
                    out=valid[:],
                    in0=hm[:],
                    scalar=0.5,
                    in1=vb,
                    op0=AluOpType.is_gt,
                    op1=AluOpType.mult,
                )

                # ascending t: last valid timestep wins
                for t in range(T):
                    mask = (
                        valid[:, :, t, :].unsqueeze(3).broadcast_to((p, f, NT, D))
                    )
                    _copy_predicated(nc.vector, ot[:], mask, xt[:, :, t, :, :])

                if mode != "compute":
                    # store on the opposite ring from this tile's x-load so it
                    # doesn't queue behind the next tile's big load
                    steng = nc.scalar if (ring != "alt" or it % 2 == 0) else nc.sync
                    steng.dma_start(
                        out=orr[i], in_=ot[:].rearrange("p f nt d -> p (f nt d)")
                    )

            if mode == "compute":
                nc.sync.dma_start(
                    out=orr[0], in_=ot[:].rearrange("p f nt d -> p (f nt d)")
                )

    nc.compile()
    return nc


_NC = None


def run_spmd(visible_treats: np.ndarray, vision: np.ndarray, **kwargs):
    global _NC
    if _NC is None:
        _NC = build_nc()
    if not kwargs.get("trace"):
        # NTFF profiling needs antenv.axon_hooks, absent in this container; a
        # stray BASS_TRACE env var would otherwise crash the run.
        import os

        os.environ.setdefault("BASS_NEVER_TRACE", "1")
    vt = np.ascontiguousarray(visible_treats, dtype=np.float32)
    vi = np.ascontiguousarray(vision, dtype=np.int32)
    in_maps = [
        {
            "x": vt[c * BC : (c + 1) * BC],
            "v": vi[c * BC : (c + 1) * BC],
        }
        for c in range(NCORES)
    ]
    return run_bass_kernel_spmd(_NC, in_maps, core_ids=list(range(NCORES)), **kwargs)


def kernel(visible_treats: np.ndarray, vision: np.ndarray) -> np.ndarray:
    res = run_spmd(visible_treats, vision)
    return np.concatenate([r["o"] for r in res.results], axis=0)



# revision 4
# speedup vs baseline: 6.4342x; 3.2117x over previous
"""Trainium2 Bass kernel for nn_BeliefModuleOld (segment_reduce).

Reference semantics per batch element b and treat type tt:
  valid[t] = (vision[b,t] != 0) and (max(visible_treats[b,t,tt,0:5]) > 0.5)
  out[b,tt,:] = visible_treats[b, last valid t, tt, :]  (or [0,0,0,0,0,1] if none)

Strategy: pure data-parallel over batch, 8 cores. Layout A: batch elements
live on SBUF partitions (P=125 used) and along the free dim (F per
partition). Per tile:
  - DMA x [P, F,5,2,6] f32 and v [P, F,5] i32 (contiguous per partition)
  - hm[t,tt] = max over d<5 of x (tensor_max tree on DVE)
  - valid = (hm > 0.5) * vision  (scalar_tensor_tensor)
  - out initialized to the default vector, then for t=0..4 ascending
    copy_predicated(out, valid[t] broadcast over d, x[t]) -- last valid wins
  - DMA out [P, F,2,6]
"""

import numpy as np

import concourse.bass as bass
import concourse.bacc as bacc
import concourse.tile as tile
from concourse import mybir
from concourse.alu_op_type import AluOpType
from concourse.bass_utils import run_bass_kernel_spmd

B, T, NT, D = 1_000_000, 5, 2, 6
NCORES = 8
BC = B // NCORES  # 125,000 per core
P = 125           # SBUF partitions used
F = 125           # batch elements per partition per tile
NTILES = BC // (P * F)  # 8 tiles, exact


def _copy_predicated(eng, out, mask, data):
    # Same as BassVectorEngine.copy_predicated but with opt=False lowering so
    # the three operand APs keep identical [p, f, nt, d] structure (the
    # broadcast mask AP cannot merge dims; unoptimized APs keep the sim's
    # np.where shapes aligned and the HW element streams in lockstep).
    return eng.add_instruction(
        mybir.InstCopyPredicated(
            name=f"I-{eng.bass.next_id()}",
            ins=[eng.lower_ap(mask, opt=False), eng.lower_ap(data, opt=False)],
            outs=[eng.lower_ap(out, opt=False)],
        )
    )


def build_nc(bc=BC, p=P, f=F, reps=1, mode="full", ring="coarsea"):
    ntiles = bc // (p * f)
    assert p * f * ntiles == bc, (bc, p, f)
    nc = bacc.Bacc("TRN2", target_bir_lowering=False)

    x = nc.dram_tensor("x", [bc, T, NT, D], mybir.dt.float32, kind="ExternalInput")
    v = nc.dram_tensor("v", [bc, T], mybir.dt.int32, kind="ExternalInput")
    o = nc.dram_tensor("o", [bc, NT, D], mybir.dt.float32, kind="ExternalOutput")

    # [ntiles, p, per-partition-contiguous block]
    xr = x[:].rearrange("(n p f) t nt d -> n p (f t nt d)", p=p, f=f)
    vr = v[:].rearrange("(n p f) t -> n p (f t)", p=p, f=f)
    orr = o[:].rearrange("(n p f) nt d -> n p (f nt d)", p=p, f=f)

    fdt = mybir.dt.float32

    if ring.startswith("rr"):
        # Round-robin WHOLE x-tile loads across nq independent DMA queues
        # (each queue sustains only ~110-130 GB/s; splitting one load's
        # partition range across queues — coarsedual — is 2x WORSE, but
        # giving different queues different complete tiles scales BW).
        # v-loads and o-stores stay on the scalar(ACT) queue.
        nq = int(ring[2:])
        engs = [nc.gpsimd, nc.sync, nc.tensor, nc.vector][:nq]
        fl = 250 if nq <= 2 else 125
        nload = bc // (p * fl)
        assert p * fl * nload == bc
        xr2 = x[:].rearrange("(n p f) t nt d -> n p (f t nt d)", p=p, f=fl)
        vr2 = v[:].rearrange("(n p f) t -> n p (f t)", p=p, f=fl)
        or2 = o[:].rearrange("(n p f) nt d -> n p (f nt d)", p=p, f=fl)
        xbufs = 2 if nq <= 2 else nq
        with tile.TileContext(nc) as tc:
            with (
                tc.tile_pool(name="xs", bufs=xbufs) as xpool,
                tc.tile_pool(name="vs", bufs=2) as vpool,
                tc.tile_pool(name="os", bufs=2) as opool,
                tc.tile_pool(name="wk", bufs=2) as wpool,
            ):
                dflt = wpool.tile([p, NT, D], fdt, tag="dflt", bufs=1)
                nc.gpsimd.memset(dflt[:, :, 0:5], 0.0)
                nc.gpsimd.memset(dflt[:, :, 5:6], 1.0)
                for it in range(reps * nload):
                    k = it % nload
                    xt = xpool.tile([p, fl, T, NT, D], fdt, tag="x")
                    engs[it % nq].dma_start(
                        out=xt[:].rearrange("p f t nt d -> p (f t nt d)"),
                        in_=xr2[k],
                    )
                    if mode == "loadrr":
                        continue
                    if mode == "dma":
                        vt = vpool.tile([p, fl, T], mybir.dt.int32, tag="v")
                        nc.scalar.dma_start(
                            out=vt[:].rearrange("p f t -> p (f t)"), in_=vr2[k]
                        )
                        nc.scalar.dma_start(
                            out=or2[k],
                            in_=xt[:].rearrange("p f t nt d -> p (f t nt d)")[
                                :, 0 : fl * NT * D
                            ],
                        )
                        continue
                    vt = vpool.tile([p, fl, T], mybir.dt.int32, tag="v")
                    ot = opool.tile([p, fl, NT, D], fdt, tag="o")
                    nc.scalar.dma_start(
                        out=vt[:].rearrange("p f t -> p (f t)"), in_=vr2[k]
                    )
                    nc.scalar.copy(
                        ot[:], dflt[:].unsqueeze(1).broadcast_to((p, fl, NT, D))
                    )
                    a = wpool.tile([p, fl, T, NT], fdt, tag="a")
                    bt = wpool.tile([p, fl, T, NT], fdt, tag="b")
                    valid = wpool.tile([p, fl, T, NT], mybir.dt.uint8, tag="valid")
                    nc.vector.tensor_max(a[:], xt[:, :, :, :, 0], xt[:, :, :, :, 1])
                    nc.vector.tensor_max(bt[:], xt[:, :, :, :, 2], xt[:, :, :, :, 3])
                    nc.vector.tensor_max(a[:], a[:], bt[:])
                    nc.vector.tensor_max(a[:], a[:], xt[:, :, :, :, 4])
                    vb = vt[:].unsqueeze(3).broadcast_to((p, fl, T, NT))
                    nc.vector.scalar_tensor_tensor(
                        out=valid[:], in0=a[:], scalar=0.5, in1=vb,
                        op0=AluOpType.is_gt, op1=AluOpType.mult,
                    )
                    for t in range(T):
                        mask = (
                            valid[:, :, t, :]
                            .unsqueeze(3)
                            .broadcast_to((p, fl, NT, D))
                        )
                        _copy_predicated(nc.vector, ot[:], mask, xt[:, :, t, :, :])
                    nc.scalar.dma_start(
                        out=or2[k], in_=ot[:].rearrange("p f nt d -> p (f nt d)")
                    )
        nc.compile()
        return nc

    if ring.startswith("coarse"):
        # One x-load feeds two compute subtiles: 4x 7.3MB loads instead of
        # 8x 3.66MB. vact ring split; in-place max tree to fit SBUF.
        # ring="coarseg": x-loads via the SWDGE (gpsimd) path instead of HWDGE.
        # ring="coarsea": coarseg + output init via ACT copy from a const tile
        # (keeps GPSIMD off the per-subtile critical path).
        xld = nc.sync if ring == "coarse" else nc.gpsimd
        f2 = 2 * f
        nload = bc // (p * f2)
        assert p * f2 * nload == bc
        xr2 = x[:].rearrange("(n p f) t nt d -> n p f t nt d", p=p, f=f2)
        vr2 = v[:].rearrange("(n p f) t -> n p (f t)", p=p, f=f2)
        or2 = o[:].rearrange("(n p f) nt d -> n p (f nt d)", p=p, f=f2)
        with tile.TileContext(nc) as tc:
            with (
                tc.tile_pool(name="xs", bufs=2) as xpool,
                tc.tile_pool(name="vs", bufs=3) as vpool,
                tc.tile_pool(name="os", bufs=3) as opool,
                tc.tile_pool(name="wk", bufs=2) as wpool,
            ):
                dflt = None
                if ring == "coarsea":
                    dflt = wpool.tile([p, NT, D], fdt, tag="dflt", bufs=1)
                    nc.gpsimd.memset(dflt[:, :, 0:5], 0.0)
                    nc.gpsimd.memset(dflt[:, :, 5:6], 1.0)
                for it in range(reps * nload * 2):
                    k, j = (it // 2) % nload, it % 2
                    if j == 0:
                        xb = xpool.tile([p, f2, T, NT, D], fdt, tag="x")
                        xbf = xb[:].rearrange("p f t nt d -> p (f t nt d)")
                        xsrc = xr2[k].rearrange("p f t nt d -> p (f t nt d)")
                        if ring == "coarsedual":
                            # halve across the two independent DGE feeders:
                            # HWDGE (sync RTL) and SWDGE (gpsimd Q7)
                            h = p // 2
                            nc.sync.dma_start(out=xbf[0:h, :], in_=xsrc[0:h, :])
                            nc.gpsimd.dma_start(out=xbf[h:p, :], in_=xsrc[h:p, :])
                        else:
                            xld.dma_start(out=xbf, in_=xsrc)
                    if mode == "dma":
                        # bench: same loads + a same-sized contiguous store
                        vt = vpool.tile([p, f, T], mybir.dt.int32, tag="v")
                        nc.scalar.dma_start(
                            out=vt[:].rearrange("p f t -> p (f t)"),
                            in_=vr2[k][:, j * f * T : (j + 1) * f * T],
                        )
                        nc.scalar.dma_start(
                            out=or2[k][:, j * f * NT * D : (j + 1) * f * NT * D],
                            in_=xb[:].rearrange("p f t nt d -> p (f t nt d)")[
                                :, 0 : f * NT * D
                            ],
                        )
                        continue
                    xt = xb[:, j * f : (j + 1) * f]
                    vt = vpool.tile([p, f, T], mybir.dt.int32, tag="v")
                    ot = opool.tile([p, f, NT, D], fdt, tag="o")
                    nc.scalar.dma_start(
                        out=vt[:].rearrange("p f t -> p (f t)"),
                        in_=vr2[k][:, j * f * T : (j + 1) * f * T],
                    )
                    if ring == "coarsea":
                        nc.scalar.copy(
                            ot[:],
                            dflt[:].unsqueeze(1).broadcast_to((p, f, NT, D)),
                        )
                    else:
                        nc.gpsimd.memset(ot[:, :, :, 0:5], 0.0)
                        nc.gpsimd.memset(ot[:, :, :, 5:6], 1.0)
                    a = wpool.tile([p, f, T, NT], fdt, tag="a")
                    bt = wpool.tile([p, f, T, NT], fdt, tag="b")
                    valid = wpool.tile([p, f, T, NT], mybir.dt.uint8, tag="valid")
                    nc.vector.tensor_max(a[:], xt[:, :, :, :, 0], xt[:, :, :, :, 1])
                    nc.vector.tensor_max(bt[:], xt[:, :, :, :, 2], xt[:, :, :, :, 3])
                    nc.vector.tensor_max(a[:], a[:], bt[:])
                    nc.vector.tensor_max(a[:], a[:], xt[:, :, :, :, 4])
                    vb = vt[:].unsqueeze(3).broadcast_to((p, f, T, NT))
                    nc.vector.scalar_tensor_tensor(
                        out=valid[:], in0=a[:], scalar=0.5, in1=vb,
                        op0=AluOpType.is_gt, op1=AluOpType.mult,
                    )
                    for t in range(T):
                        mask = (
                            valid[:, :, t, :]
                            .unsqueeze(3)
                            .broadcast_to((p, f, NT, D))
                        )
                        _copy_predicated(nc.vector, ot[:], mask, xt[:, :, t, :, :])
                    nc.scalar.dma_start(
                        out=or2[k][:, j * f * NT * D : (j + 1) * f * NT * D],
                        in_=ot[:].rearrange("p f nt d -> p (f nt d)"),
                    )
        nc.compile()
        return nc

    if mode.startswith("bw"):
        # pure-load BW sweep: [128, q*240B] tiles via one engine.
        # mode="bw<q>" (q batch elems per partition), ring: g/a/s engine.
        q = int(mode[2:])
        ce = q * T * NT * D  # f32 elems per partition
        per_load = 128 * ce * 4
        nld = max(2, round(30_000_000 / per_load))
        nld = min(nld, bc // (128 * q))
        eng = {"g": nc.gpsimd, "a": nc.scalar, "s": nc.sync}[ring[0]]
        xfl = x[0 : nld * 128 * q].rearrange(
            "(n p q) t nt d -> n p (q t nt d)", p=128, q=q
        )
        bufs = 2 if ce * 4 <= 96 * 1024 else 1
        with tile.TileContext(nc) as tc:
            with tc.tile_pool(name="xs", bufs=bufs) as xpool:
                for it in range(reps * nld):
                    i = it % nld
                    xt = xpool.tile([128, ce], fdt, tag="x")
                    eng.dma_start(out=xt[:], in_=xfl[i])
        nc.compile()
        print(f"bw mode: {nld} loads x {per_load} B = {nld*per_load/1e6:.1f} MB/rep")
        return nc

    if mode.startswith("load128"):
        # pure-load microbench: x as [128, chunk] tiles from the flat region
        q = 244 if mode == "load128big" else 122
        n128 = (bc // (128 * q))
        xl = x[0 : n128 * 128 * q].rearrange(
            "(n p q) t nt d -> n p (q t nt d)", p=128, q=q
        )
        with tile.TileContext(nc) as tc:
            with tc.tile_pool(name="xs", bufs=2) as xpool:
                for it in range(reps * n128):
                    i = it % n128
                    xt = xpool.tile([128, q * T * NT * D], fdt, tag="x")
                    nc.sync.dma_start(out=xt[:], in_=xl[i])
        nc.compile()
        return nc

    with tile.TileContext(nc) as tc:
        with (
            tc.tile_pool(name="xs", bufs=3) as xpool,
            tc.tile_pool(name="vs", bufs=3) as vpool,
            tc.tile_pool(name="os", bufs=3) as opool,
            tc.tile_pool(name="wk", bufs=2) as wpool,
        ):
            vbig = None
            if ring == "vpre":
                # preload ALL vision data once (2.5 MB) so the steady state
                # has only the big x-loads and the output stores
                vbig = vpool.tile([p, ntiles, f, T], mybir.dt.int32, tag="vbig")
                nc.scalar.dma_start(
                    out=vbig[:].rearrange("p n f t -> p n (f t)"),
                    in_=v[:].rearrange("(n p f) t -> p n (f t)", p=p, f=f),
                )
            if mode == "compute":
                # bench mode: load one tile, run the compute chain reps*ntiles
                # times on resident tiles, store once.
                xt = xpool.tile([p, f, T, NT, D], fdt, tag="x")
                vt = vpool.tile([p, f, T], mybir.dt.int32, tag="v")
                ot = opool.tile([p, f, NT, D], fdt, tag="o")
                nc.sync.dma_start(
                    out=xt[:].rearrange("p f t nt d -> p (f t nt d)"), in_=xr[0]
                )
                nc.sync.dma_start(out=vt[:].rearrange("p f t -> p (f t)"), in_=vr[0])

            for it, i in enumerate(
                [i for _ in range(reps) for i in range(ntiles)]
            ):
                if mode != "compute":
                    xt = xpool.tile([p, f, T, NT, D], fdt, tag="x")
                    ot = opool.tile([p, f, NT, D], fdt, tag="o")
                    # ring="fixed": loads on SP ring, stores on ACT ring.
                    # ring="alt": alternate per tile. ring="split": halve the
                    # x-load across both rings. ring="vact": v-load on ACT.
                    # ring="vpre": vision preloaded once before the loop.
                    xtf = xt[:].rearrange("p f t nt d -> p (f t nt d)")
                    if ring == "vpre":
                        vt = None
                    else:
                        vt = vpool.tile([p, f, T], mybir.dt.int32, tag="v")
                        vtf = vt[:].rearrange("p f t -> p (f t)")
                    if ring == "split":
                        h = p // 2
                        nc.sync.dma_start(out=xtf[0:h, :], in_=xr[i][0:h, :])
                        nc.scalar.dma_start(out=xtf[h:p, :], in_=xr[i][h:p, :])
                        nc.sync.dma_start(out=vtf, in_=vr[i])
                    else:
                        ldeng = (
                            nc.sync if (ring != "alt" or it % 2 == 0) else nc.scalar
                        )
                        ldeng.dma_start(out=xtf, in_=xr[i])
                        if vt is not None:
                            veng = nc.scalar if ring == "vact" else nc.sync
                            veng.dma_start(out=vtf, in_=vr[i])
                if mode == "dma":
                    # bench mode: DMA traffic only; store a contiguous chunk
                    # of the x tile with the same shape as the real output
                    xflat = xt[:].rearrange("p f t nt d -> p (f t nt d)")
                    nc.scalar.dma_start(out=orr[i], in_=xflat[:, 0 : f * NT * D])
                    continue
                if mode == "load":
                    continue  # bench mode: loads only

                # out = default = [0,0,0,0,0,1]
                nc.gpsimd.memset(ot[:, :, :, 0:5], 0.0)
                nc.gpsimd.memset(ot[:, :, :, 5:6], 1.0)

                a = wpool.tile([p, f, T, NT], fdt, tag="a")
                bt = wpool.tile([p, f, T, NT], fdt, tag="b")
                c = wpool.tile([p, f, T, NT], fdt, tag="c")
                hm = wpool.tile([p, f, T, NT], fdt, tag="hm")
                # uint8: walrus requires an integer mask dtype for CopyPredicated
                valid = wpool.tile([p, f, T, NT], mybir.dt.uint8, tag="valid")

                nc.vector.tensor_max(a[:], xt[:, :, :, :, 0], xt[:, :, :, :, 1])
                nc.vector.tensor_max(bt[:], xt[:, :, :, :, 2], xt[:, :, :, :, 3])
                nc.vector.tensor_max(c[:], a[:], bt[:])
                nc.vector.tensor_max(hm[:], c[:], xt[:, :, :, :, 4])

                # valid = (hm > 0.5) * vision, vision broadcast over tt
                # (DVE: walrus rejects TensorScalarPtr/TensorTensor on Pool)
                if ring == "vpre" and mode != "compute":
                    vb = vbig[:, i, :, :].unsqueeze(3).broadcast_to((p, f, T, NT))
                else:
                    vb = vt[:].unsqueeze(3).broadcast_to((p, f, T, NT))
                nc.vector.scalar_tensor_tensor(
                    out=valid[:],
                    in0=hm[:],
                    scalar=0.5,
                    in1=vb,
                    op0=AluOpType.is_gt,
                    op1=AluOpType.mult,
                )

                # ascending t: last valid timestep wins
                for t in range(T):
                    mask = (
                        valid[:, :, t, :].unsqueeze(3).broadcast_to((p, f, NT, D))
                    )
                    _copy_predicated(nc.vector, ot[:], mask, xt[:, :, t, :, :])

                if mode != "compute":
                    # store on the opposite ring from this tile's x-load so it
                    # doesn't queue behind the next tile's big load
                    steng = nc.scalar if (ring != "alt" or it % 2 == 0) else nc.sync
                    steng.dma_start(
                        out=orr[i], in_=ot[:].rearrange("p f nt d -> p (f nt d)")
                    )

            if mode == "compute":
                nc.sync.dma_start(
                    out=orr[0], in_=ot[:].rearrange("p f nt d -> p (f nt d)")
                )

    nc.compile()
    return nc


_NC = None


def run_spmd(visible_treats: np.ndarray, vision: np.ndarray, **kwargs):
    global _NC
    if _NC is None:
        _NC = build_nc()
    if not kwargs.get("trace"):
        # NTFF profiling needs antenv.axon_hooks, absent in this container; a
        # stray BASS_TRACE env var would otherwise crash the run.
        import os

        os.environ.setdefault("BASS_NEVER_TRACE", "1")
    vt = np.ascontiguousarray(visible_treats, dtype=np.float32)
    vi = np.ascontiguousarray(vision, dtype=np.int32)
    in_maps = [
        {
            "x": vt[c * BC : (c + 1) * BC],
            "v": vi[c * BC : (c + 1) * BC],
        }
        for c in range(NCORES)
    ]
    return run_bass_kernel_spmd(_NC, in_maps, core_ids=list(range(NCORES)), **kwargs)


def kernel(visible_treats: np.ndarray, vision: np.ndarray) -> np.ndarray:
    res = run_spmd(visible_treats, vision)
    return np.concatenate([r["o"] for r in res.results], axis=0)

